# revision 1
# baseline (speedup 1.0000x reference)
"""Trainium2 Bass kernel for a 2-attention-block + FFN decoder stack.

Shapes: x (4, 2048, 768), 12 heads x 64, d_ff 3072.
Sharding over 8 cores: core c handles batch b=c//2 and heads 6*(c%2)..+6 for
both attention blocks; the final FFN+LN runs on token half c%2 of batch b.
Two 8-rank AllGathers exchange the per-head attention outputs so each core
can LayerNorm over the full model dim.

All compute is done in "transposed" layout (D on partitions, tokens on the
free axis).  The source model's softmax runs over the *query* axis (dim=2
quirk), which in transposed layout (k on partitions, q on free axis) is a
per-partition-row softmax: exp on ScalarE with accum_out produces the row
sums for free; the 1/rowsum is folded into the (tiny) KV matrix instead of
the (huge) score matrix.  No max-subtraction is needed: |w| stays O(10) so
exp cannot overflow, and softmax is shift-invariant.
"""

import os
import sys

for _p in ("/opt/trn_rl_repo", "/root/.axon_site/_ro/trn_rl_repo"):
    if os.path.isdir(_p) and _p not in sys.path:
        sys.path.insert(0, _p)

import numpy as np
from contextlib import ExitStack

from concourse import bass, bacc, mybir, tile
from concourse import bass_utils

F32 = mybir.dt.float32
BF16 = mybir.dt.bfloat16
I32 = mybir.dt.int32
NP_BF16 = mybir.dt.np(BF16)

B, S, D, H, DH, DFF = 4, 2048, 768, 12, 64, 3072
NCORES = 8
HLOC = 6           # heads per core
NPAIR = 3          # head pairs per core
SQRT_DK = float(np.sqrt(DH))
EPS = 1e-5
SH = S // 2        # token half for FFN
DT = D // 128      # 6 d-tiles
KT = S // 128      # 16 k-tiles
QC = S // 512      # 4 q-chunks
FT = DFF // 128    # 24 ff-tiles

Exp = mybir.ActivationFunctionType.Exp
Sqrt = mybir.ActivationFunctionType.Sqrt
Add = mybir.AluOpType.add
Mult = mybir.AluOpType.mult
Max = mybir.AluOpType.max


def _scalar_from_input(nc, dram, max_val):
    tmp = nc.alloc_registers(f"sv_{dram.name}", mybir.ALL_ENGINES)
    nc.regs_load(tmp, dram[0:1, 0:1])
    return nc.snap(tmp, donate=True, min_val=0, max_val=max_val)


def _layernorm(tc, ctx, r_tiles, gb_sb, ones_f, ones_b, width, out_f, out_b):
    """LayerNorm over the partition (D) axis of 6 x (128, width) f32 tiles.

    out_f gets the normalized f32 tiles; out_b (optional) bf16 copies.
    gb_sb is a (2, 768) f32 SBUF tile (row 0 gamma, row 1 beta), applied via
    tiny outer-product matmuls building per-element affine maps.
    """
    nc = tc.nc
    ch_n = width // 512
    sb = ctx.enter_context(tc.tile_pool(name="ln_sb", bufs=1))
    # all row-vector scratch lives at base partition 0 (engine requirement)
    mu = sb.tile([1, width], F32, tag="ln_mu", name="ln_mu")
    msq = sb.tile([1, width], F32, tag="ln_msq", name="ln_msq")
    am = sb.tile([1, width], F32, tag="ln_am", name="ln_am")
    bm = sb.tile([2, width], F32, tag="ln_bm", name="ln_bm")
    nc.vector.memset(bm[0:2, :], 1.0)  # row1 stays ones; row0 overwritten

    with ExitStack() as sctx:
        sq_pool = sctx.enter_context(tc.tile_pool(name="ln_sq", bufs=2))
        ps_pool = sctx.enter_context(
            tc.tile_pool(name="ln_stats_ps", bufs=1, space="PSUM"))
        sum_ps = [ps_pool.tile([1, 512], F32, tag=f"sum{ch}", name=f"sum{ch}")
                  for ch in range(ch_n)]
        ssq_ps = [ps_pool.tile([1, 512], F32, tag=f"ssq{ch}", name=f"ssq{ch}")
                  for ch in range(ch_n)]
        for dt in range(DT):
            rb = sq_pool.tile([128, width], BF16, tag="rb", name="rb")
            nc.vector.tensor_copy(rb[:], r_tiles[dt][:])
            sq = sq_pool.tile([128, width], BF16, tag="sq", name="sq")
            nc.vector.tensor_mul(sq[:], rb[:], rb[:])
            for ch in range(ch_n):
                cs = slice(512 * ch, 512 * ch + 512)
                nc.tensor.matmul(sum_ps[ch][:], ones_b[:, 0:1],
                                 rb[:, cs],
                                 start=(dt == 0), stop=(dt == DT - 1))
                nc.tensor.matmul(ssq_ps[ch][:], ones_b[:, 0:1],
                                 sq[:, cs],
                                 start=(dt == 0), stop=(dt == DT - 1))
        for ch in range(ch_n):
            cs = slice(512 * ch, 512 * ch + 512)
            nc.vector.tensor_scalar_mul(mu[0:1, cs], sum_ps[ch][:], 1.0 / D)
            nc.vector.tensor_scalar_mul(msq[0:1, cs], ssq_ps[ch][:], 1.0 / D)

    # var = msq - mu^2 ; sd = sqrt(var + eps) ; rstd = 1/sd ; -mu*rstd
    # chunked so early chunks' broadcast matmuls start before late chunks'
    # stats finish (cuts the serial row-chain latency out of the LN span)
    tmp = sb.tile([1, width], F32, tag="ln_tmp", name="ln_tmp")
    amb = sb.tile([1, width], BF16, tag="ln_amb", name="ln_amb")
    bmb = sb.tile([2, width], BF16, tag="ln_bmb", name="ln_bmb")
    for ch in range(ch_n):
        cs = slice(512 * ch, 512 * ch + 512)
        nc.vector.tensor_mul(tmp[0:1, cs], mu[0:1, cs], mu[0:1, cs])
        nc.vector.tensor_sub(msq[0:1, cs], msq[0:1, cs], tmp[0:1, cs])
        nc.vector.tensor_scalar_add(msq[0:1, cs], msq[0:1, cs], EPS)
        nc.scalar.activation(msq[0:1, cs], msq[0:1, cs], Sqrt)
        nc.vector.reciprocal(am[0:1, cs], msq[0:1, cs])
        nc.vector.scalar_tensor_tensor(bm[0:1, cs], mu[0:1, cs], -1.0,
                                       am[0:1, cs], op0=Mult, op1=Mult)
        nc.vector.tensor_copy(amb[0:1, cs], am[0:1, cs])
        nc.vector.tensor_copy(bmb[0:2, cs], bm[0:2, cs])

    with tc.tile_pool(name="ln_ab_ps", bufs=1, space="PSUM") as ab_pool:
        for dt in range(DT):
            amat = ab_pool.tile([128, width], F32, tag="ln_amat", name="ln_amat")
            bmat = ab_pool.tile([128, width], F32, tag="ln_bmat", name="ln_bmat")
            for ch in range(ch_n):
                cs = slice(512 * ch, 512 * ch + 512)
                nc.tensor.matmul(amat[:, cs],
                                 gb_sb[0:1, 128 * dt:128 * dt + 128],
                                 amb[0:1, cs], start=True, stop=True)
                nc.tensor.matmul(bmat[:, cs],
                                 gb_sb[0:2, 128 * dt:128 * dt + 128],
                                 bmb[0:2, cs], start=True, stop=True)
            nc.vector.tensor_mul(out_f[dt][:], r_tiles[dt][:], amat[:])
            nc.vector.tensor_add(out_f[dt][:], out_f[dt][:], bmat[:])
            if out_b is not None:
                nc.vector.tensor_copy(out_b[dt][:], out_f[dt][:])


def _attention(tc, ctx, x_tiles, x_dram, wq_dram, wv_dram, mask_dram,
               ag_in, ident_sb, on_pair=None):
    """One attention block in transposed layout (all-bf16 matmul operands).

    x input either as x_tiles (6 x (128, S) bf16 SBUF, caller-owned) or
    x_dram (bf16, loaded into a projection-scoped pool, freed afterwards).
    mask_tiles: None or 16 x (128, S) bf16 additive-mask tiles ((k, q)
    layout); applied by accumulating an identity-matmul of the mask into the
    attention-weight PSUM (keeps the DVE out of the softmax path entirely).
    Writes o^T for this core's 6 heads (384, S) f32 into ag_in DRAM.
    """
    nc = tc.nc

    qkv_pool = ctx.enter_context(tc.tile_pool(name="attn_qkv", bufs=NPAIR))
    kv_pool = ctx.enter_context(tc.tile_pool(name="attn_kv", bufs=KT))
    mask_pool = None
    if mask_dram is not None:
        mask_pool = ctx.enter_context(tc.tile_pool(name="attn_mask", bufs=KT))
    qt_sb, kvt_sb, kv_sb = [], [], []

    with ExitStack() as proj_ctx:
        wpool = proj_ctx.enter_context(tc.tile_pool(name="attn_w", bufs=DT))
        wq_sb, wv_sb = [], []
        for dt in range(DT):
            wq = wpool.tile([128, HLOC * DH], BF16, tag="wq", name="wq")
            nc.sync.dma_start(wq[:], wq_dram[128 * dt:128 * dt + 128, :])
            wq_sb.append(wq)
        for dt in range(DT):
            wv = wpool.tile([128, HLOC * DH], BF16, tag="wv", name="wv")
            nc.sync.dma_start(wv[:], wv_dram[128 * dt:128 * dt + 128, :])
            wv_sb.append(wv)
        if x_tiles is None:
            xp = proj_ctx.enter_context(tc.tile_pool(name="attn_x", bufs=DT))
            x_tiles = []
            for dt in range(DT):
                t = xp.tile([128, S], BF16, tag="x", name="x")
                nc.sync.dma_start(t[:], x_dram[128 * dt:128 * dt + 128, :])
                x_tiles.append(t)

        with tc.tile_pool(name="attn_proj_ps", bufs=2, space="PSUM") as pps:
            for p in range(NPAIR):
                for which, wsb, dst in (("q", wq_sb, qt_sb),
                                        ("v", wv_sb, kvt_sb)):
                    ps = pps.tile([128, S], F32, tag="proj", name="proj")
                    for qc in range(QC):
                        cs = slice(512 * qc, 512 * qc + 512)
                        for dt in range(DT):
                            nc.tensor.matmul(
                                ps[:, cs], wsb[dt][:, 128 * p:128 * p + 128],
                                x_tiles[dt][:, cs],
                                start=(dt == 0), stop=(dt == DT - 1))
                    out = qkv_pool.tile([128, S], BF16, tag=f"qkv_{which}",
                                        name=f"qkv_{which}")
                    nc.vector.tensor_copy(out[:], ps[:])
                    dst.append(out)
        with tc.tile_pool(name="attn_kvtok_ps", bufs=6, space="PSUM") as kps:
            for kt in range(KT):
                ps = kps.tile([128, HLOC * DH], F32, tag="kvtok", name="kvtok")
                for dt in range(DT):
                    nc.tensor.matmul(ps[:],
                                     x_tiles[dt][:, 128 * kt:128 * kt + 128],
                                     wv_sb[dt][:],
                                     start=(dt == 0), stop=(dt == DT - 1))
                kv = kv_pool.tile([128, HLOC * DH], BF16, tag="kv", name="kv")
                nc.vector.tensor_copy(kv[:], ps[:])
                kv_sb.append(kv)

    mask_tiles = None
    if mask_dram is not None:
        mask_tiles = []
        for kt in range(KT):
            m = mask_pool.tile([128, S], BF16, tag="mask", name="mask")
            nc.sync.dma_start(m[:], mask_dram[128 * kt:128 * kt + 128, :])
            mask_tiles.append(m)

    # attention proper, one head-pair at a time.
    # PSUM: ot (128,2048)f32 = 4 banks; wt (128,1024)f32 x 2 bufs = 4 banks.
    with (
        tc.tile_pool(name="attn_wt_ps", bufs=2, space="PSUM") as wt_pool,
        tc.tile_pool(name="attn_ot_ps", bufs=1, space="PSUM") as ot_pool,
        tc.tile_pool(name="attn_sc", bufs=4) as sc_pool,
        tc.tile_pool(name="attn_rs", bufs=8) as rs_pool,
        tc.tile_pool(name="attn_o", bufs=3) as o_pool,
    ):
        for p in range(NPAIR):
            ot = ot_pool.tile([128, S], F32, tag="ot", name="ot")
            for kt in range(KT):
                ksl = slice(128 * kt, 128 * kt + 128)
                heads = {}
                for hi, (plo, phi) in enumerate(((0, 64), (64, 128))):
                    score = sc_pool.tile([128, S], BF16, tag=f"sc{hi}",
                                         name=f"sc{hi}")
                    rsh = rs_pool.tile([128, 2], F32, tag=f"rsh{hi}",
                                       name=f"rsh{hi}")
                    for half in range(2):
                        wt = wt_pool.tile([128, 1024], F32, tag="wt",
                                          name="wt")
                        for q2 in range(2):
                            qoff = 1024 * half + 512 * q2
                            qs = slice(qoff, qoff + 512)
                            ws = slice(512 * q2, 512 * q2 + 512)
                            nc.tensor.matmul(wt[:, ws],
                                             kvt_sb[p][plo:phi, ksl],
                                             qt_sb[p][plo:phi, qs],
                                             start=True,
                                             stop=(mask_tiles is None),
                                             tile_position=(plo, 0))
                            if mask_tiles is not None:
                                nc.tensor.matmul(wt[:, ws], ident_sb[:],
                                                 mask_tiles[kt][:, qs],
                                                 start=False, stop=True)
                        nc.scalar.activation(
                            score[:, 1024 * half:1024 * half + 1024], wt[:],
                            Exp, accum_out=rsh[:, half:half + 1])
                    rs = rs_pool.tile([128, 1], F32, tag=f"rs{hi}",
                                      name=f"rs{hi}")
                    nc.vector.tensor_add(rs[:], rsh[:, 0:1], rsh[:, 1:2])
                    ri = rs_pool.tile([128, 1], F32, tag=f"ri{hi}",
                                      name=f"ri{hi}")
                    nc.vector.reciprocal(ri[:], rs[:])
                    kvs = rs_pool.tile([128, DH], BF16, tag=f"kvs{hi}",
                                       name=f"kvs{hi}")
                    h_local = 2 * p + hi
                    nc.vector.tensor_scalar_mul(
                        kvs[:], kv_sb[kt][:, DH * h_local:DH * h_local + DH],
                        ri[:])
                    heads[hi] = (score, kvs)
                for hi, (plo, phi) in enumerate(((0, 64), (64, 128))):
                    score, kvs = heads[hi]
                    for qc in range(QC):
                        cs = slice(512 * qc, 512 * qc + 512)
                        nc.tensor.matmul(ot[plo:phi, cs], kvs[:],
                                         score[:, cs],
                                         start=(kt == 0), stop=(kt == KT - 1),
                                         tile_position=(0, plo))
            o_sb = o_pool.tile([128, S], F32, tag="o", name="o")
            nc.vector.tensor_copy(o_sb[:], ot[:])
            nc.sync.dma_start(ag_in[128 * p:128 * p + 128, :], o_sb[:])
            if on_pair is not None:
                on_pair(p)


def build(nc, stage="full", reps=1):
    xT = nc.dram_tensor("xT", [D, S], F32, kind="ExternalInput")
    maskT = nc.dram_tensor("maskT", [S, S], BF16, kind="ExternalInput")
    xTb = nc.dram_tensor("xTb", [D, S], BF16, kind="ExternalInput")
    ident = nc.dram_tensor("ident", [128, 128], BF16, kind="ExternalInput")
    wq1 = nc.dram_tensor("wq1", [D, HLOC * DH], BF16, kind="ExternalInput")
    wv1 = nc.dram_tensor("wv1", [D, HLOC * DH], BF16, kind="ExternalInput")
    wq2 = nc.dram_tensor("wq2", [D, HLOC * DH], BF16, kind="ExternalInput")
    wv2 = nc.dram_tensor("wv2", [D, HLOC * DH], BF16, kind="ExternalInput")
    w1 = nc.dram_tensor("w1", [D, DFF], BF16, kind="ExternalInput")
    w2 = nc.dram_tensor("w2", [DFF, D], BF16, kind="ExternalInput")
    b1c = nc.dram_tensor("b1c", [DFF, 1], F32, kind="ExternalInput")
    b2c = nc.dram_tensor("b2c", [D, 1], F32, kind="ExternalInput")
    gb1 = nc.dram_tensor("gb1", [2, D], BF16, kind="ExternalInput")
    gb2 = nc.dram_tensor("gb2", [2, D], BF16, kind="ExternalInput")
    gbf = nc.dram_tensor("gbf", [2, D], BF16, kind="ExternalInput")
    rb = nc.dram_tensor("rb", [1, 1], I32, kind="ExternalInput")
    cb = nc.dram_tensor("cb", [1, 1], I32, kind="ExternalInput")

    ag1_in = nc.dram_tensor("ag1_in", [NPAIR * 128, S], F32)
    ag1_outs = [nc.dram_tensor(f"ag1_out{p}", [NCORES * 128, S], F32,
                               addr_space="Shared") for p in range(NPAIR)]
    x2s = nc.dram_tensor("x2s", [D, S], F32)
    ag2_in = nc.dram_tensor("ag2_in", [NPAIR * 128, S], F32)
    ag2_outs = [nc.dram_tensor(f"ag2_out{p}", [NCORES * 128, S], F32,
                               addr_space="Shared") for p in range(NPAIR)]

    if stage in ("x2", "b1", "b1nm"):
        dbg = nc.dram_tensor("dbg", [D, S], F32, kind="ExternalOutput")
    elif stage == "x3":
        dbg = nc.dram_tensor("dbg", [D, SH], F32, kind="ExternalOutput")
    outT = None
    if stage in ("full", "sim"):
        outT = nc.dram_tensor("outT", [D, SH], F32, kind="ExternalOutput")

    rg = [list(range(NCORES))]

    with tile.TileContext(nc) as tc:
        rv = _scalar_from_input(nc, rb, 256 * (B - 1))
        cv = _scalar_from_input(nc, cb, SH)
        for _rep in range(reps):
            _build_body(tc, nc, stage, rv, cv, locals())


def _all_gather_pair(nc, stage, rg, ag_in, ag_out_p, p):
    """AllGather one head-pair's slice (emitted as soon as pair p's o^T is
    in DRAM, so earlier pairs' exchange overlaps later pairs' compute)."""
    in_ap = ag_in[128 * p:128 * p + 128, :]
    if stage.startswith("sim"):
        nc.sync.dma_start(ag_out_p[0:128, :], in_ap)
        nc.sync.dma_start(ag_out_p[128:256, :], in_ap)
    else:
        nc.gpsimd.collective_compute(
            "AllGather", mybir.AluOpType.bypass, replica_groups=rg,
            ins=[in_ap.opt()], outs=[ag_out_p[:].opt()])


def _build_body(tc, nc, stage, rv, cv, env):
    (xT, maskT, wq1, wv1, wq2, wv2, w1, w2, b1c, b2c, gb1, gb2, gbf,
     x2s, ag1_in, ag1_outs, ag2_in, ag2_outs, rg, xTb, ident) = (
        env["xT"], env["maskT"], env["wq1"], env["wv1"], env["wq2"],
        env["wv2"], env["w1"], env["w2"], env["b1c"], env["b2c"],
        env["gb1"], env["gb2"], env["gbf"], env["x2s"], env["ag1_in"],
        env["ag1_outs"], env["ag2_in"], env["ag2_outs"], env["rg"],
        env["xTb"], env["ident"])
    dbg = env.get("dbg")
    outT = env.get("outT")
    with ExitStack() as top:
        const_pool = top.enter_context(tc.tile_pool(name="const", bufs=1))
        ones_f = const_pool.tile([128, 1], F32, tag="ones_f", name="ones_f")
        ones_b = const_pool.tile([128, 1], BF16, tag="ones_b", name="ones_b")
        nc.vector.memset(ones_f[:], 1.0)
        nc.vector.memset(ones_b[:], 1.0)
        gb_sb = {}
        for nm, dram in (("gb1", gb1), ("gb2", gb2), ("gbf", gbf)):
            t = const_pool.tile([2, D], BF16, tag=nm, name=nm)
            nc.sync.dma_start(t[:], dram[:])
            gb_sb[nm] = t
        ident_sb = const_pool.tile([128, 128], BF16, tag="ident", name="ident")
        nc.sync.dma_start(ident_sb[:], ident[:])

        # ---------------- block 1 ----------------
        with ExitStack() as blk1:
            _attention(tc, blk1, None, xTb, wq1, wv1,
                       None if stage == "b1nm" else maskT, ag1_in,
                       ident_sb,
                       on_pair=lambda p: _all_gather_pair(
                           nc, stage, rg, ag1_in, ag1_outs[p], p))

        if stage in ("b1", "b1nm"):
            with tc.tile_pool(name="b1dbg", bufs=2) as dp:
                for dt in range(DT):
                    t = dp.tile([128, S], F32, tag="d", name="d")
                    nc.sync.dma_start(
                        t[:], ag1_outs[dt % NPAIR][
                            128 * (dt // NPAIR):128 * (dt // NPAIR) + 128, :])
                    nc.sync.dma_start(dbg[128 * dt:128 * dt + 128, :], t[:])
            return

        # ---------------- LN1 -> x2 ; block 2 ----------------
        with ExitStack() as x2scope:
            x2_pool = x2scope.enter_context(tc.tile_pool(name="x2", bufs=DT))
            x2f = [x2_pool.tile([128, S], F32, tag="x2f", name="x2f")
                   for _ in range(DT)]
            with ExitStack() as lctx:
                rp = lctx.enter_context(tc.tile_pool(name="ln1_r", bufs=DT))
                tp = lctx.enter_context(tc.tile_pool(name="ln1_t", bufs=2))
                r_tiles = []
                for dt in range(DT):
                    t1 = tp.tile([128, S], F32, tag="ag", name="ag")
                    t2 = tp.tile([128, S], F32, tag="xres", name="xres")
                    nc.sync.dma_start(
                        t1[:], ag1_outs[dt % NPAIR][
                            bass.ds(rv + 128 * (dt // NPAIR), 128), :])
                    nc.sync.dma_start(t2[:], xT[128 * dt:128 * dt + 128, :])
                    r = rp.tile([128, S], F32, tag="r", name="r")
                    nc.vector.tensor_add(r[:], t1[:], t2[:])
                    r_tiles.append(r)
                _layernorm(tc, lctx, r_tiles, gb_sb["gb1"], ones_f, ones_b, S,
                           x2f, None)
            # spill x2 for the LN2 residual read-back (frees SBUF for FFN)
            for dt in range(DT):
                nc.sync.dma_start(x2s[128 * dt:128 * dt + 128, :], x2f[dt][:])

            if stage == "x2":
                for dt in range(DT):
                    nc.sync.dma_start(dbg[128 * dt:128 * dt + 128, :],
                                      x2f[dt][:])
                return

            # ---------------- block 2 ----------------
            with ExitStack() as blk2:
                x2b_pool = blk2.enter_context(
                    tc.tile_pool(name="x2b", bufs=DT))
                x2b = [x2b_pool.tile([128, S], BF16, tag="x2b", name="x2b")
                       for _ in range(DT)]
                for dt in range(DT):
                    nc.vector.tensor_copy(x2b[dt][:], x2f[dt][:])
                _attention(tc, blk2, x2b, None, wq2, wv2, None, ag2_in,
                           ident_sb,
                           on_pair=lambda p: _all_gather_pair(
                               nc, stage, rg, ag2_in, ag2_outs[p], p))

        # prefetch FFN weights/biases during the LN2 window
        ffn_stack = None
        if stage in ("full", "sim"):
            x3_pool = top.enter_context(tc.tile_pool(name="x3", bufs=DT))
            r3_pool = top.enter_context(tc.tile_pool(name="r3", bufs=DT))
            ffn_stack = ExitStack()
            b_pool = ffn_stack.enter_context(tc.tile_pool(name="ffn_b", bufs=1))
            b1_sb, b2_sb = [], []
            for ft in range(FT):
                bt = b_pool.tile([128, 1], F32, tag=f"b1_{ft}", name=f"b1_{ft}")
                nc.sync.dma_start(bt[:], b1c[128 * ft:128 * ft + 128, :])
                b1_sb.append(bt)
            for dt in range(DT):
                bt = b_pool.tile([128, 1], F32, tag=f"b2_{dt}", name=f"b2_{dt}")
                nc.sync.dma_start(bt[:], b2c[128 * dt:128 * dt + 128, :])
                b2_sb.append(bt)
            w1_pool = ffn_stack.enter_context(
                tc.tile_pool(name="ffn_w1", bufs=DT))
            w1_sb = []
            for dt in range(DT):
                wt = w1_pool.tile([128, DFF], BF16, tag="w1", name="w1")
                nc.sync.dma_start(wt[:], w1[128 * dt:128 * dt + 128, :])
                w1_sb.append(wt)
        # ---------------- LN2 -> x3 (token half) ----------------
        if stage == "x3":
            x3_pool = top.enter_context(tc.tile_pool(name="x3", bufs=DT))
        x3f = [x3_pool.tile([128, SH], F32, tag="x3f", name="x3f") for _ in range(DT)]
        x3b = [x3_pool.tile([128, SH], BF16, tag="x3b", name="x3b") for _ in range(DT)]
        with ExitStack() as lctx:
            rp = lctx.enter_context(tc.tile_pool(name="ln2_r", bufs=DT))
            tp = lctx.enter_context(tc.tile_pool(name="ln2_t", bufs=2))
            r_tiles = []
            for dt in range(DT):
                t1 = tp.tile([128, SH], F32, tag="ag", name="ag")
                t2 = tp.tile([128, SH], F32, tag="xres", name="xres")
                nc.sync.dma_start(
                    t1[:], ag2_outs[dt % NPAIR][
                        bass.ds(rv + 128 * (dt // NPAIR), 128),
                        bass.ds(cv, SH)])
                nc.sync.dma_start(
                    t2[:], x2s[128 * dt:128 * dt + 128, bass.ds(cv, SH)])
                r = rp.tile([128, SH], F32, tag="r", name="r")
                nc.vector.tensor_add(r[:], t1[:], t2[:])
                r_tiles.append(r)
            _layernorm(tc, lctx, r_tiles, gb_sb["gb2"], ones_f, ones_b, SH,
                       x3f, x3b)

        if stage == "x3":
            for dt in range(DT):
                nc.sync.dma_start(dbg[128 * dt:128 * dt + 128, :], x3f[dt][:])
            return

        # ---------------- FFN ----------------
        r3 = [r3_pool.tile([128, SH], F32, tag="r3", name="r3")
              for _ in range(DT)]
        with ffn_stack:
            w2_pool = ffn_stack.enter_context(
                tc.tile_pool(name="ffn_w2", bufs=FT))
            w2_sb = []
            for ft in range(FT):
                wt = w2_pool.tile([128, D], BF16, tag="w2", name="w2")
                nc.sync.dma_start(wt[:], w2[128 * ft:128 * ft + 128, :])
                w2_sb.append(wt)
            h_pool = ffn_stack.enter_context(tc.tile_pool(name="ffn_h", bufs=3))
            with (
                tc.tile_pool(name="ffn_h_ps", bufs=2, space="PSUM") as hps,
                tc.tile_pool(name="ffn_y_ps", bufs=1, space="PSUM") as yps,
            ):
                for ch in range(SH // 512):
                    cs = slice(512 * ch, 512 * ch + 512)
                    y_ps = [yps.tile([128, 512], F32, tag=f"yp{dt}",
                                     name=f"yp{dt}") for dt in range(DT)]
                    for ft in range(FT):
                        ps = hps.tile([128, 512], F32, tag="hp", name="hp")
                        for dt in range(DT):
                            nc.tensor.matmul(
                                ps[:], w1_sb[dt][:, 128 * ft:128 * ft + 128],
                                x3b[dt][:, cs],
                                start=(dt == 0), stop=(dt == DT - 1))
                        h = h_pool.tile([128, 512], BF16, tag="h", name="h")
                        nc.vector.tensor_scalar(h[:], ps[:], b1_sb[ft][:],
                                                0.0, op0=Add, op1=Max)
                        for dt in range(DT):
                            nc.tensor.matmul(
                                y_ps[dt][:],
                                w2_sb[ft][:, 128 * dt:128 * dt + 128],
                                h[:],
                                start=(ft == 0), stop=(ft == FT - 1))
                    for dt in range(DT):
                        nc.vector.scalar_tensor_tensor(
                            r3[dt][:, cs], y_ps[dt][:], b2_sb[dt][:],
                            x3f[dt][:, cs], op0=Add, op1=Add)

        # ---------------- LN3 -> out ----------------
        with ExitStack() as lctx:
            ofin = [r3_pool.tile([128, SH], F32, tag="ofin", name="ofin")
                    for _ in range(DT)]
            _layernorm(tc, lctx, r3, gb_sb["gbf"], ones_f, ones_b, SH,
                       ofin, None)
            for dt in range(DT):
                nc.sync.dma_start(outT[128 * dt:128 * dt + 128, :],
                                  ofin[dt][:])


_CACHE = {}


def _get_compiled(stage="full"):
    if stage not in _CACHE:
        reps = 1
        name = stage
        import re as _re
        m = _re.match(r"^(.*)_r(\d+)$", stage)
        if m:
            name, reps = m.group(1), int(m.group(2))
        ndev = 1 if name.startswith("sim") else NCORES
        nc = bacc.Bacc("TRN2", target_bir_lowering=False, debug=False,
                       num_devices=ndev)
        build(nc, name, reps=reps)
        nc.compile()
        _CACHE[stage] = nc
    return _CACHE[stage]


def make_in_maps(x, mask, Wq1, Wv1, g1, be1, Wq2, Wv2, g2, be2,
                 Wf1, bf1, Wf2, bf2, gf, bef):
    x = np.asarray(x, np.float32)
    mask = np.asarray(mask)
    maskT = np.where(np.asarray(mask[0, 0]).T, np.float32(-1e9),
                     np.float32(0.0)).astype(NP_BF16)
    w1b = np.asarray(Wf1, np.float32).astype(NP_BF16)
    w2b = np.asarray(Wf2, np.float32).astype(NP_BF16)
    scale = np.float32(1.0 / SQRT_DK)
    in_maps = []
    for c in range(NCORES):
        b, hh = c // 2, c % 2
        cols = slice(HLOC * DH * hh, HLOC * DH * (hh + 1))
        xTf = np.ascontiguousarray(x[b].T)
        in_maps.append({
            "xT": xTf,
            "xTb": xTf.astype(NP_BF16),
            "ident": np.eye(128, dtype=np.float32).astype(NP_BF16),
            "maskT": maskT,
            # fold the 1/sqrt(dk) into the Q projection
            "wq1": (np.ascontiguousarray(
                np.asarray(Wq1, np.float32)[:, cols]) * scale).astype(NP_BF16),
            "wv1": np.ascontiguousarray(
                np.asarray(Wv1, np.float32)[:, cols]).astype(NP_BF16),
            "wq2": (np.ascontiguousarray(
                np.asarray(Wq2, np.float32)[:, cols]) * scale).astype(NP_BF16),
            "wv2": np.ascontiguousarray(
                np.asarray(Wv2, np.float32)[:, cols]).astype(NP_BF16),
            "w1": w1b,
            "w2": w2b,
            "b1c": np.asarray(bf1, np.float32).reshape(DFF, 1),
            "b2c": np.asarray(bf2, np.float32).reshape(D, 1),
            "gb1": np.stack([np.asarray(g1, np.float32),
                             np.asarray(be1, np.float32)]).astype(NP_BF16),
            "gb2": np.stack([np.asarray(g2, np.float32),
                             np.asarray(be2, np.float32)]).astype(NP_BF16),
            "gbf": np.stack([np.asarray(gf, np.float32),
                             np.asarray(bef, np.float32)]).astype(NP_BF16),
            "rb": np.array([[256 * b]], np.int32),
            "cb": np.array([[SH * hh]], np.int32),
        })
    return in_maps


def run_spmd(in_maps, stage="full"):
    nc = _get_compiled(stage)
    return bass_utils.run_bass_kernel_spmd(nc, in_maps,
                                           core_ids=list(range(NCORES)))


def kernel(**inputs):
    in_maps = make_in_maps(**inputs)
    res = run_spmd(in_maps, "full")
    out = np.empty((B, S, D), np.float32)
    for c in range(NCORES):
        b, hh = c // 2, c % 2
        out[b, SH * hh:SH * (hh + 1), :] = res.results[c]["outT"].T
    return out


class _Runner:
    """Reusable jitted dispatcher (mirrors bass2jax.run_bass_via_pjrt's
    multi-core path) so repeated executions skip re-tracing and host
    transfers — used for timing."""

    def __init__(self, stage="full"):
        import jax
        from jax.sharding import Mesh, PartitionSpec
        from jax.experimental.shard_map import shard_map
        from concourse import bass2jax as b2j

        b2j.install_neuronx_cc_hook()
        nc = _get_compiled(stage)
        pname = (nc.partition_id_tensor.name
                 if nc.partition_id_tensor else None)
        in_names, out_names, out_avals = [], [], []
        for alloc in nc.m.functions[0].allocations:
            if not isinstance(alloc, mybir.MemoryLocationSet):
                continue
            name = alloc.memorylocations[0].name
            if alloc.kind == "ExternalInput":
                if name != pname:
                    in_names.append(name)
            elif alloc.kind == "ExternalOutput":
                out_names.append(name)
                out_avals.append(jax.core.ShapedArray(
                    tuple(alloc.tensor_shape), mybir.dt.np(alloc.dtype)))
        self.in_names, self.out_names = list(in_names), list(out_names)
        self.out_avals = out_avals
        all_in = in_names + out_names
        if pname is not None:
            all_in = all_in + [pname]
        n_params, n_outs = len(in_names), len(out_names)

        def _body(*args):
            operands = list(args)
            if pname is not None:
                operands.append(b2j.partition_id_tensor())
            outs = b2j._bass_exec_p.bind(
                *operands, out_avals=tuple(out_avals), in_names=tuple(all_in),
                out_names=tuple(out_names), lowering_input_output_aliases=(),
                sim_require_finite=True, sim_require_nnan=True, nc=nc)
            return tuple(outs)

        devices = jax.devices()[:NCORES]
        mesh = Mesh(np.asarray(devices), ("core",))
        in_specs = (PartitionSpec("core"),) * (n_params + n_outs)
        out_specs = (PartitionSpec("core"),) * n_outs
        self.fn = jax.jit(
            shard_map(_body, mesh=mesh, in_specs=in_specs,
                      out_specs=out_specs, check_rep=False),
            donate_argnums=tuple(range(n_params, n_params + n_outs)),
            keep_unused=True)
        self._jax = jax

    def device_inputs(self, in_maps):
        import jax
        concat = [np.concatenate([np.asarray(in_maps[c][n])
                                  for c in range(NCORES)], axis=0)
                  for n in self.in_names]
        return [jax.device_put(a) for a in concat]

    def zero_outs(self):
        import jax.numpy as jnp
        return [jnp.zeros((NCORES * av.shape[0], *av.shape[1:]), av.dtype)
                for av in self.out_avals]

    def __call__(self, dev_in, zeros):
        return self.fn(*dev_in, *zeros)


class _RunnerNZ:
    """Timing runner: zero output buffers are created inside the shard_map
    body (device-local), so repeated calls move no host data at all."""

    def __init__(self, stage="full"):
        import jax
        import jax.numpy as jnp
        from jax.sharding import Mesh, PartitionSpec
        from jax.experimental.shard_map import shard_map
        from concourse import bass2jax as b2j

        b2j.install_neuronx_cc_hook()
        nc = _get_compiled(stage)
        pname = (nc.partition_id_tensor.name
                 if nc.partition_id_tensor else None)
        in_names, out_names, out_avals = [], [], []
        for alloc in nc.m.functions[0].allocations:
            if not isinstance(alloc, mybir.MemoryLocationSet):
                continue
            name = alloc.memorylocations[0].name
            if alloc.kind == "ExternalInput":
                if name != pname:
                    in_names.append(name)
            elif alloc.kind == "ExternalOutput":
                out_names.append(name)
                out_avals.append(jax.core.ShapedArray(
                    tuple(alloc.tensor_shape), mybir.dt.np(alloc.dtype)))
        self.in_names, self.out_names = in_names, out_names
        all_in = in_names + out_names
        if pname is not None:
            all_in = all_in + [pname]

        def _body(*args):
            operands = list(args)
            operands += [jnp.zeros(av.shape, av.dtype) for av in out_avals]
            if pname is not None:
                operands.append(b2j.partition_id_tensor())
            outs = b2j._bass_exec_p.bind(
                *operands, out_avals=tuple(out_avals), in_names=tuple(all_in),
                out_names=tuple(out_names), lowering_input_output_aliases=(),
                sim_require_finite=True, sim_require_nnan=True, nc=nc)
            return tuple(outs)

        devices = jax.devices()[:NCORES]
        mesh = Mesh(np.asarray(devices), ("core",))
        self.fn = jax.jit(
            shard_map(_body, mesh=mesh,
                      in_specs=(PartitionSpec("core"),) * len(in_names),
                      out_specs=(PartitionSpec("core"),) * len(out_names),
                      check_rep=False),
            keep_unused=True)

    def device_inputs(self, in_maps):
        import jax
        concat = [np.concatenate([np.asarray(in_maps[c][n])
                                  for c in range(NCORES)], axis=0)
                  for n in self.in_names]
        return [jax.device_put(a) for a in concat]

    def __call__(self, dev_in):
        return self.fn(*dev_in)

